# revision 1
# baseline (speedup 1.0000x reference)
"""Trainium2 Bass kernel for nn_AttentionBlock (B=2, C=256, D=8, H=32, W=32).

reference math:
    xf = x.reshape(B, C, N)                        # N = 8192
    q = wq @ xf + bq                               # (B, 32, N)
    k = wk @ xf + bk                               # (B, 32, N)
    v = wv @ xf + bv                               # (B, 256, N)
    attn = softmax(q^T k, axis=-1)                 # (B, N, N)
    out = attn @ v^T                               # (B, N, C) buffer
    result = gamma * out.reshape(B, C, D, H, W) + x

Sharding (8 cores): core i -> batch b = i//4, query-chunk c = i%4 of 2048
rows.  Each core gets its batch's full xf (for K/V), a host-sliced xq for
its Q rows, and the matching flat residual slice.  No collectives.

Device algorithm per core (scores are tiny, |S| < ~5, so softmax is computed
without max-subtraction):
    out = (exp(S) @ [vT | 1]) ; rows normalized by the appended ones-column
S is computed transposed (keys on partitions, queries on free dim) via
4x row-packed K=32 matmuls; exp runs on ScalarE PSUM->SBUF (bf16);
attn@V accumulates f32 in PSUM over 64 key tiles; the epilogue fuses
*1/rowsum + residual into the PSUM copyback.  gamma is folded into wv/bv
on the host.
"""

import numpy as np

B, C, Dd, Hh, Ww = 2, 256, 8, 32, 32
N = Dd * Hh * Ww          # 8192
CQK = C // 8              # 32
NCORES = 8
QCHUNK = N // 4           # 2048 query rows per core
P = 128

PACK_S = True             # 4x row-packed K=32 score matmuls


def build_graph(n=N, nq=QCHUNK):
    import concourse.bass as bass
    import concourse.tile as tile
    from concourse import bacc, mybir
    from concourse.bass import ds, ts

    f32 = mybir.dt.float32
    bf16 = mybir.dt.bfloat16
    AF = mybir.ActivationFunctionType

    n_t = n // 512            # 512-wide column tiles of xf
    nq_t = nq // 512          # 512-wide column tiles of xq
    m_tiles = n // P          # 128-wide key tiles
    m_supers = n // 512       # groups of 4 key tiles
    n_sc = nq // 512          # query subchunks
    cin_o = C // P            # 2

    nc = bacc.Bacc()
    xf_d = nc.declare_dram_parameter("xf", [C, n], bf16, isOutput=False)
    xq_d = nc.declare_dram_parameter("xq", [C, nq], bf16, isOutput=False)
    xres_d = nc.declare_dram_parameter("xres", [nq, C], f32, isOutput=False)
    wqT_d = nc.declare_dram_parameter("wqT", [C, CQK], bf16, isOutput=False)
    wkT_d = nc.declare_dram_parameter("wkT", [C, CQK], bf16, isOutput=False)
    wvT_d = nc.declare_dram_parameter("wvT", [C, C], bf16, isOutput=False)
    bq_d = nc.declare_dram_parameter("bq", [CQK, 1], f32, isOutput=False)
    bk_d = nc.declare_dram_parameter("bk", [CQK, 1], f32, isOutput=False)
    bv_d = nc.declare_dram_parameter("bv", [P, C], f32, isOutput=False)
    out_d = nc.declare_dram_parameter("out", [nq, C], f32, isOutput=True)

    with tile.TileContext(nc) as tc:
        with tc.tile_pool(name="singles", bufs=1) as singles, \
             tc.tile_pool(name="ostage", bufs=3) as ostage, \
             tc.tile_pool(name="small", bufs=4) as small, \
             tc.tile_pool(name="ptp", bufs=3) as ptp:

            # ---- constants / weights -------------------------------------
            wqT_s = singles.tile([P, cin_o, CQK], bf16)
            wkT_s = singles.tile([P, cin_o, CQK], bf16)
            wvT_s = singles.tile([P, cin_o, C], bf16)
            for d, sb in ((wqT_d, wqT_s), (wkT_d, wkT_s), (wvT_d, wvT_s)):
                nc.gpsimd.dma_start(out=sb[:], in_=d[:].rearrange(
                    "(co p) m -> p co m", p=P))

            bq_s = singles.tile([P, 1], f32)
            bk_s = singles.tile([P, 1], f32)
            nc.gpsimd.dma_start(out=bq_s[0:CQK, :], in_=bq_d[:])
            nc.gpsimd.dma_start(out=bk_s[0:CQK, :], in_=bk_d[:])
            bv_s = singles.tile([P, C], f32)
            nc.gpsimd.dma_start(out=bv_s, in_=bv_d[:])

            xres_s = singles.tile([P, nq // P, C], f32)
            nc.gpsimd.dma_start(out=xres_s, in_=xres_d[:].rearrange(
                "(t p) c -> p t c", p=P))

            # ---- load xf, xq (bf16, cast on host) ------------------------
            xf_bf = singles.tile([P, cin_o, n], bf16)
            xfr = xf_d[:].rearrange("(co p) m -> p co m", p=P)
            for t in range(4):
                nc.gpsimd.dma_start(out=xf_bf[:, :, ts(t, n // 4)],
                                    in_=xfr[:, :, ts(t, n // 4)])
            xq_bf = singles.tile([P, cin_o, nq], bf16)
            xqr = xq_d[:].rearrange("(co p) m -> p co m", p=P)
            for t in range(2):
                nc.gpsimd.dma_start(out=xq_bf[:, :, ts(t, nq // 2)],
                                    in_=xqr[:, :, ts(t, nq // 2)])

            # ---- projections ---------------------------------------------
            k_rep = singles.tile([P, m_supers, 512], bf16)
            q_rep = singles.tile([P, n_sc, 512], bf16)
            vT = singles.tile([P, m_tiles, C + 1], bf16)
            nc.vector.memset(vT[:, :, C:C + 1], 1.0)

            with tc.tile_pool(name="pp", bufs=2, space="PSUM") as pp:
                # k (all n columns), written to partition group 0 of k_rep
                for t in range(n_t):
                    ps_k = pp.tile([P, 512], f32, tag="psk", name="ps_k")
                    for co in range(cin_o):
                        nc.tensor.matmul(
                            ps_k[0:CQK, :], lhsT=wkT_s[:, co, :],
                            rhs=xf_bf[:, co, ts(t, 512)],
                            start=(co == 0), stop=(co == cin_o - 1))
                    nc.scalar.activation(
                        k_rep[0:CQK, t, :], ps_k[0:CQK, :], AF.Identity,
                        bias=bk_s[0:CQK, :])
                # q (nq columns only)
                for t in range(nq_t):
                    ps_q = pp.tile([P, 512], f32, tag="psk", name="ps_q")
                    for co in range(cin_o):
                        nc.tensor.matmul(
                            ps_q[0:CQK, :], lhsT=wqT_s[:, co, :],
                            rhs=xq_bf[:, co, ts(t, 512)],
                            start=(co == 0), stop=(co == cin_o - 1))
                    nc.scalar.activation(
                        q_rep[0:CQK, t, :], ps_q[0:CQK, :], AF.Identity,
                        bias=bq_s[0:CQK, :])
                # replicate k, q to partition groups 1..3 (SBUF->SBUF DMA)
                for j in range(1, 4):
                    nc.gpsimd.dma_start(out=k_rep[ds(32 * j, 32), :, :],
                                      in_=k_rep[0:32, :, :])
                    nc.gpsimd.dma_start(out=q_rep[ds(32 * j, 32), :, :],
                                      in_=q_rep[0:32, :, :])
                # vT[m, c] = sum_cin xf[cin, m] * wvT[cin, c]  (+ bv)
                for m in range(m_tiles):
                    ps_v = pp.tile([P, C], f32, tag="psv", name="ps_v")
                    for co in range(cin_o):
                        nc.tensor.matmul(
                            ps_v, lhsT=xf_bf[:, co, ts(m, P)],
                            rhs=wvT_s[:, co, :],
                            start=(co == 0), stop=(co == cin_o - 1))
                    nc.vector.tensor_add(vT[:, m, 0:C], ps_v, bv_s)

            # ---- attention ------------------------------------------------
            outr = out_d[:].rearrange("(t p) c -> p t c", p=P)
            with tc.tile_pool(name="stp", bufs=1, space="PSUM") as stp, \
                 tc.tile_pool(name="op", bufs=1, space="PSUM") as op:
                for sc in range(n_sc):
                    out_ps = [op.tile([P, C + 1], f32, tag=f"ops{qt}",
                                      name=f"out_ps{qt}")
                              for qt in range(4)]
                    for ms in range(m_supers):
                        pT = ptp.tile([P, 4, 512], bf16, tag="pt", name="pT")
                        for j in range(4):
                            st_ps = stp.tile([P, 512], f32, tag=f"st{j}",
                                             name=f"st_ps{j}")
                            if PACK_S:
                                nc.tensor.matmul(
                                    st_ps,
                                    lhsT=k_rep[ds(32 * j, 32), ms, ts(j, P)],
                                    rhs=q_rep[ds(32 * j, 32), sc, :],
                                    start=True, stop=True,
                                    tile_position=(32 * j, 0))
                            else:
                                nc.tensor.matmul(
                                    st_ps,
                                    lhsT=k_rep[0:32, ms, ts(j, P)],
                                    rhs=q_rep[0:32, sc, :],
                                    start=True, stop=True)
                            nc.scalar.activation(pT[:, j, :], st_ps, AF.Exp)
                        for qt in range(4):
                            for j in range(4):
                                nc.tensor.matmul(
                                    out_ps[qt],
                                    lhsT=pT[:, j, ts(qt, P)],
                                    rhs=vT[:, 4 * ms + j, :],
                                    start=(ms == 0 and j == 0),
                                    stop=(ms == m_supers - 1 and j == 3))
                    # epilogue: out = psum[:, :C] / rowsum + xres
                    for qt in range(4):
                        rec = small.tile([P, 1], f32, tag="rec", name="rec")
                        nc.vector.reciprocal(rec, out_ps[qt][:, C:C + 1])
                        ot = ostage.tile([P, C], f32, tag="ot", name="ot")
                        nc.vector.tensor_scalar_mul(ot, out_ps[qt][:, 0:C], rec)
                        nc.vector.tensor_add(ot, ot, xres_s[:, 4 * sc + qt, :])
                        nc.gpsimd.dma_start(out=outr[:, 4 * sc + qt, :], in_=ot)
    nc.compile()
    return nc


_nc_cache = {}


def _get_graph(n=N, nq=QCHUNK):
    key = (n, nq)
    if key not in _nc_cache:
        _nc_cache[key] = build_graph(n, nq)
    return _nc_cache[key]


def _make_in_maps(x, wq, bq, wk, bk, wv, bv, gamma, n=N, nq=QCHUNK):
    import ml_dtypes
    bf = ml_dtypes.bfloat16
    xf = np.ascontiguousarray(x.reshape(B, C, n)).astype(np.float32)
    xf16 = xf.astype(bf)
    g = float(np.asarray(gamma).reshape(-1)[0])
    wqT = np.ascontiguousarray(np.asarray(wq, dtype=np.float32).T).astype(bf)
    wkT = np.ascontiguousarray(np.asarray(wk, dtype=np.float32).T).astype(bf)
    wvT = np.ascontiguousarray(
        (g * np.asarray(wv, dtype=np.float32)).T).astype(bf)
    bq2 = np.asarray(bq, dtype=np.float32).reshape(CQK, 1)
    bk2 = np.asarray(bk, dtype=np.float32).reshape(CQK, 1)
    bvr = np.ascontiguousarray(
        np.tile((g * np.asarray(bv, dtype=np.float32))[None, :], (P, 1)))
    nchunks = n // nq
    in_maps = []
    for i in range(NCORES):
        b, c = divmod(i, nchunks)
        n0 = c * nq
        in_maps.append({
            "xf": xf16[b],
            "xq": np.ascontiguousarray(xf16[b][:, n0:n0 + nq]),
            "xres": xf[b].reshape(-1)[n0 * C:(n0 + nq) * C]
                    .reshape(nq, C).copy(),
            "wqT": wqT, "wkT": wkT, "wvT": wvT,
            "bq": bq2, "bk": bk2, "bv": bvr,
        })
    return in_maps


def _assemble(results, n=N, nq=QCHUNK):
    nchunks = n // nq
    outs = []
    for b in range(B):
        buf = np.concatenate(
            [results[b * nchunks + c]["out"] for c in range(nchunks)], axis=0)
        outs.append(buf.reshape(C, Dd, Hh, Ww))
    return np.stack(outs).astype(np.float32)


def kernel(x, wq, bq, wk, bk, wv, bv, gamma):
    from concourse.bass_utils import run_bass_kernel_spmd
    nc = _get_graph()
    in_maps = _make_in_maps(x, wq, bq, wk, bk, wv, bv, gamma)
    res = run_bass_kernel_spmd(nc, in_maps, core_ids=list(range(NCORES)))
    return _assemble(res.results)

